# revision 1
# baseline (speedup 1.0000x reference)
"""Causal self-attention (shared score matrix) on 8 Trainium2 cores.

Strategy: sequence-parallel. Core r owns query rows [512r, 512(r+1)).
  Phase 1: project own rows -> qT (SBUF resident), kT + v -> local DRAM.
  Phase 2: AllGather kT, then v, across the 8 cores.
  Phase 3: scoresT[s,t] = KT . qT per 128-row s-tile; exp on ACT;
           causal mask applied as (col_iota >= thr_sigma) * exp in one
           fused DVE op (thr is per-core input data -> one SPMD program).
  Rowsum:  matmuls vs a ones block; reciprocal on DVE.
  Phase 4: outUT[e,t] = V . attnT (unnormalized).
  Phase 5: y[t,e2] = (outU @ W_out) * recip[t] + b_eff, fused in DVE.

All matmuls run in float32r (fp32 with 11-bit mantissa, full PE rate at
N>=256; HW rounds inputs RTN) accumulating in fp32 PSUM.

Host folds b_v into b_eff = b_v @ W_out + b_out (exact because softmax
rows sum to 1), pre-transposes x, pre-tiles W_qk into lhsT-friendly
blocks, and reassembles y from the 8 shards.

DMA staging is blocked for contiguity: per-partition runs are >=1KB
(kT rank blocks 2KB, V mu-pair blocks 1KB, W_qk m-pair blocks 1KB).
W_v and W_out are read exactly once via 8-bank PSUM half-column
blocking.
"""

import numpy as np

from concourse import bacc, mybir, tile
from concourse.bass_utils import run_bass_kernel_spmd
from concourse.bass_interp import get_hw_module

N_CORES = 8
S = 4096          # seq len
E = 2048          # embed dim
SL = S // N_CORES  # 512 rows per core
P = 128
NE = E // P       # 16 e-tiles
NS = S // P       # 32 s-tiles
NM = SL // P      # 4 t-tiles per core
SCALE = 1.0 / np.sqrt(128.0)  # 1/sqrt(head_dim)
EH = E // 2       # 1024: half embed for single-read W_v / W_out passes

F32 = mybir.dt.float32
F32R = mybir.dt.float32r

_CACHE = {}


def _build(single=False, do_ag=True, do_p3=True, do_p4=True, do_p5=True,
           reps=1):
    """single=True: collective-free 1-core variant for TimelineSim (AG
    outputs become ExternalInputs). do_* flags truncate phases for HW
    timing bisection; a dummy y write keeps the output produced."""
    do_p4 = do_p4 and do_p3
    do_p5 = do_p5 and do_p4

    nc = bacc.Bacc("TRN2", target_bir_lowering=False, debug=False,
                   num_devices=1 if single else N_CORES)

    xT = nc.dram_tensor("xT", [E, SL], F32R, kind="ExternalInput")
    # W_qk pre-tiled on host: [16 g][16 k][128 p][256 c]; group g holds
    # m-tiles (2g, 2g+1); c spans their two 128-col blocks.
    w_qk = nc.dram_tensor("w_qk", [16, NE, P, 256], F32R,
                          kind="ExternalInput")
    w_v = nc.dram_tensor("w_v", [E, E], F32R, kind="ExternalInput")
    w_out = nc.dram_tensor("w_out", [E, E], F32R, kind="ExternalInput")
    b_qk = nc.dram_tensor("b_qk", [P, 2 * NE], F32, kind="ExternalInput")
    b_eff = nc.dram_tensor("b_eff", [P, E], F32, kind="ExternalInput")
    col_iota = nc.dram_tensor("col_iota", [P, SL], F32, kind="ExternalInput")
    ones = nc.dram_tensor("ones", [P, 256], F32R, kind="ExternalInput")
    thr = nc.dram_tensor("thr", [P, NS], F32, kind="ExternalInput")
    y_ext = nc.dram_tensor("y", [SL, E], F32, kind="ExternalOutput")

    EXP = mybir.ActivationFunctionType.Exp
    IDENT = mybir.ActivationFunctionType.Identity
    GE = mybir.AluOpType.is_ge
    MULT = mybir.AluOpType.mult
    ADD = mybir.AluOpType.add

    with tile.TileContext(nc) as tc:
        for _rep in range(reps):
            with (
                tc.tile_pool(name="consts", bufs=1) as consts,
                tc.tile_pool(name="dram", bufs=1, space="DRAM") as dram,
            ):
                b_qk_sb = consts.tile([P, 2 * NE], F32)
                col_iota_sb = consts.tile([P, SL], F32)
                thr_sb = consts.tile([P, NS], F32)
                ones_sb = consts.tile([P, 256], F32R)
                recip_sb = consts.tile([P, NM], F32)
                nc.sync.dma_start(b_qk_sb[:], b_qk.ap()[:])
                nc.sync.dma_start(col_iota_sb[:], col_iota.ap()[:])
                nc.sync.dma_start(thr_sb[:], thr.ap()[:])
                nc.sync.dma_start(ones_sb[:], ones.ap()[:])

                kT_loc = dram.tile([E, SL], F32R)
                v_loc = dram.tile([SL, E], F32R)
                if single:
                    kT_all = nc.dram_tensor(
                        f"kT_all{_rep}", [N_CORES, E, SL], F32R,
                        kind="ExternalInput").ap()
                    v_all = nc.dram_tensor(
                        f"v_all{_rep}", [N_CORES, SL, E], F32R,
                        kind="ExternalInput").ap()
                else:
                    kT_all = dram.tile([N_CORES, E, SL], F32R,
                                       addr_space="Shared")
                    v_all = dram.tile([N_CORES, SL, E], F32R,
                                      addr_space="Shared")

                sc_ctx = tc.tile_pool(name="sc", bufs=1)
                sc_pool = sc_ctx.__enter__()
                scores_sb = sc_pool.tile([P, NS * SL], F32R,
                                         name="scores_sb")
                qt_ctx = tc.tile_pool(name="qt", bufs=1)
                qt_pool = qt_ctx.__enter__()
                qT_sb = qt_pool.tile([P, NE * SL], F32R, name="qT_sb")

                # ---------------- Phase 1: projections ----------------
                with tc.tile_pool(name="p1", bufs=2) as p1:
                    xT_sb = p1.tile([P, NE * SL], F32R, name="xT_sb",
                                    bufs=1)
                    nc.sync.dma_start(
                        xT_sb[:].rearrange("p (k t) -> p k t", k=NE),
                        xT.ap().rearrange("(k p) t -> p k t", p=P))

                    # K projection first (g 8..15), then AG(kT), then Q.
                    with tc.tile_pool(name="p1ps", bufs=4,
                                      space="PSUM") as p1ps:
                        def qk_proj_pair(g):
                            """Project m-tiles (2g, 2g+1); returns psums."""
                            wcol2 = p1.tile([P, NE * 256], F32R,
                                            tag="wcol2", name="wcol2")
                            nc.sync.dma_start(
                                wcol2[:].rearrange("p (k c) -> p k c",
                                                   k=NE),
                                w_qk.ap()[g].rearrange("k p c -> p k c"))
                            out = []
                            for ml in range(2):
                                ps = p1ps.tile([P, SL], F32, tag="qkps",
                                               name="qkps")
                                for k in range(NE):
                                    nc.tensor.matmul(
                                        ps[:],
                                        wcol2[:, k * 256 + ml * P:
                                              k * 256 + (ml + 1) * P],
                                        xT_sb[:, k * SL:(k + 1) * SL],
                                        start=(k == 0), stop=(k == NE - 1))
                                out.append(ps)
                            return out

                        for g in range(8, 16):  # K projection
                            pss = qk_proj_pair(g)
                            for ml in range(2):
                                m = 2 * g + ml
                                km = m - NE
                                ktmp = p1.tile([P, SL], F32R, tag="ktmp",
                                               name="ktmp")
                                nc.scalar.activation(
                                    ktmp[:], pss[ml][:], IDENT,
                                    bias=b_qk_sb[:, m:m + 1], scale=1.0)
                                nc.sync.dma_start(
                                    kT_loc[km * P:(km + 1) * P, :],
                                    ktmp[:])

                        if not single and do_ag:
                            nc.gpsimd.collective_compute(
                                "AllGather", mybir.AluOpType.bypass,
                                replica_groups=[list(range(N_CORES))],
                                ins=[kT_loc.opt()], outs=[kT_all.opt()])

                        for g in range(8):  # Q projection -> qT_sb
                            pss = qk_proj_pair(g)
                            for ml in range(2):
                                m = 2 * g + ml
                                nc.scalar.activation(
                                    qT_sb[:, m * SL:(m + 1) * SL],
                                    pss[ml][:], IDENT,
                                    bias=b_qk_sb[:, m:m + 1], scale=1.0)

                    # V projection, single read of w_v via half-columns:
                    # psum [128, 4096] = 4 t-tiles x 1024 cols = 8 banks.
                    with tc.tile_pool(name="p1vps", bufs=1,
                                      space="PSUM") as p1vps:
                        for eh in range(2):
                            vps = p1vps.tile([P, NM * EH], F32, tag="vps",
                                             name="vps")
                            for k in range(NE):
                                wvh = p1.tile([P, EH], F32R, tag="wvh",
                                              name="wvh")
                                nc.sync.dma_start(
                                    wvh[:],
                                    w_v.ap()[k * P:(k + 1) * P,
                                             eh * EH:(eh + 1) * EH])
                                for m in range(NM):
                                    for n in range(2):
                                        nc.tensor.matmul(
                                            vps[:, m * EH + n * SL:
                                                m * EH + (n + 1) * SL],
                                            xT_sb[:, k * SL + m * P:
                                                  k * SL + (m + 1) * P],
                                            wvh[:, n * SL:(n + 1) * SL],
                                            start=(k == 0),
                                            stop=(k == NE - 1))
                            for m in range(NM):
                                vtmp = p1.tile([P, EH], F32R, tag="vtmp",
                                               name="vtmp")
                                nc.vector.tensor_copy(
                                    vtmp[:], vps[:, m * EH:(m + 1) * EH])
                                nc.sync.dma_start(
                                    v_loc[m * P:(m + 1) * P,
                                          eh * EH:(eh + 1) * EH],
                                    vtmp[:])

                    if not single and do_ag:
                        nc.gpsimd.collective_compute(
                            "AllGather", mybir.AluOpType.bypass,
                            replica_groups=[list(range(N_CORES))],
                            ins=[v_loc.opt()], outs=[v_all.opt()])

                # ------------- Phase 3: scoresT + exp + mask -------------
                if do_p3:
                    with (
                        tc.tile_pool(name="p3", bufs=2) as p3,
                        tc.tile_pool(name="p3ps", bufs=4,
                                     space="PSUM") as p3ps,
                    ):
                        for r in range(N_CORES):
                            # whole rank block of KT: [2048 e, 512 t],
                            # per-partition runs of 2KB.
                            kstage = p3.tile([P, NE * SL], F32R,
                                             tag="kstage", name="kstage")
                            nc.sync.dma_start(
                                kstage[:].rearrange("p (k t) -> p k t",
                                                    k=NE),
                                kT_all[r].rearrange("(k p) t -> p k t",
                                                    p=P))
                            for j in range(4):
                                sg = 4 * r + j
                                ps = p3ps.tile([P, SL], F32, tag="scps",
                                               name="scps")
                                for k in range(NE):
                                    nc.tensor.matmul(
                                        ps[:],
                                        kstage[:, k * SL + j * P:
                                               k * SL + (j + 1) * P],
                                        qT_sb[:, k * SL:(k + 1) * SL],
                                        start=(k == 0),
                                        stop=(k == NE - 1))
                                etmp = p3.tile([P, SL], F32, tag="etmp",
                                               name="etmp")
                                nc.scalar.activation(etmp[:], ps[:], EXP,
                                                     bias=0.0,
                                                     scale=float(SCALE))
                                nc.vector.scalar_tensor_tensor(
                                    scores_sb[:, sg * SL:(sg + 1) * SL],
                                    col_iota_sb[:], thr_sb[:, sg:sg + 1],
                                    etmp[:], op0=GE, op1=MULT)

                    # Rowsum + reciprocal
                    with tc.tile_pool(name="rsps", bufs=2,
                                      space="PSUM") as rsps:
                        for m in range(NM):
                            rs = rsps.tile([P, 256], F32, tag="rs",
                                           name="rs")
                            for sg in range(NS):
                                nc.tensor.matmul(
                                    rs[:],
                                    scores_sb[:, sg * SL + m * P:
                                              sg * SL + (m + 1) * P],
                                    ones_sb[:],
                                    start=(sg == 0), stop=(sg == NS - 1))
                            nc.vector.reciprocal(recip_sb[:, m:m + 1],
                                                 rs[:, 0:1])

                qt_ctx.__exit__(None, None, None)  # free qT (LIFO ok)

                # ------------- Phase 4: outUT = V . attnT -------------
                if do_p4:
                    ot_ctx = tc.tile_pool(name="ot", bufs=1)
                    ot_pool = ot_ctx.__enter__()
                    outUT_sb = ot_pool.tile([P, NE * SL], F32R,
                                            name="outUT_sb")
                    with (
                        tc.tile_pool(name="p4", bufs=2) as p4,
                        tc.tile_pool(name="p4ps", bufs=4,
                                     space="PSUM") as p4ps,
                    ):
                        for g in range(8):  # mu-pair groups
                            # V cols [256] for all 4096 s; runs of 1KB.
                            vstage = p4.tile([P, NS * 256], F32R,
                                             tag="vstage", name="vstage")
                            nc.sync.dma_start(
                                vstage[:].rearrange(
                                    "p (r j c) -> p r j c",
                                    r=N_CORES, j=4),
                                v_all[:, :, g * 256:(g + 1) * 256]
                                .rearrange("r (j p) c -> p r j c", p=P))
                            for ml in range(2):
                                mu = 2 * g + ml
                                ps = p4ps.tile([P, SL], F32, tag="o4ps",
                                               name="o4ps")
                                for sg in range(NS):
                                    nc.tensor.matmul(
                                        ps[:],
                                        vstage[:, sg * 256 + ml * P:
                                               sg * 256 + (ml + 1) * P],
                                        scores_sb[:, sg * SL:
                                                  (sg + 1) * SL],
                                        start=(sg == 0),
                                        stop=(sg == NS - 1))
                                nc.vector.tensor_copy(
                                    outUT_sb[:, mu * SL:(mu + 1) * SL],
                                    ps[:])

                # ------------- Phase 5: y = outU @ W_out -------------
                if do_p5:
                    with (
                        tc.tile_pool(name="p5", bufs=2) as p5,
                        tc.tile_pool(name="p5ps", bufs=1,
                                     space="PSUM") as p5ps,
                    ):
                        b_eff_sb = consts.tile([P, E], F32,
                                               name="b_eff_sb")
                        nc.sync.dma_start(b_eff_sb[:], b_eff.ap()[:])
                        for eh in range(2):
                            zps = p5ps.tile([P, NM * EH], F32, tag="zps",
                                            name="zps")
                            for k in range(NE):
                                woh = p5.tile([P, EH], F32R, tag="woh",
                                              name="woh")
                                nc.sync.dma_start(
                                    woh[:],
                                    w_out.ap()[k * P:(k + 1) * P,
                                               eh * EH:(eh + 1) * EH])
                                for m in range(NM):
                                    for n in range(2):
                                        nc.tensor.matmul(
                                            zps[:, m * EH + n * SL:
                                                m * EH + (n + 1) * SL],
                                            outUT_sb[:, k * SL + m * P:
                                                     k * SL + (m + 1) * P],
                                            woh[:, n * SL:(n + 1) * SL],
                                            start=(k == 0),
                                            stop=(k == NE - 1))
                            for m in range(NM):
                                y_sb = p5.tile([P, EH], F32, tag="ysb",
                                               name="ysb")
                                nc.vector.scalar_tensor_tensor(
                                    y_sb[:],
                                    zps[:, m * EH:(m + 1) * EH],
                                    recip_sb[:, m:m + 1],
                                    b_eff_sb[:, eh * EH:(eh + 1) * EH],
                                    op0=MULT, op1=ADD)
                                nc.sync.dma_start(
                                    y_ext.ap()[m * P:(m + 1) * P,
                                               eh * EH:(eh + 1) * EH],
                                    y_sb[:])
                else:
                    with tc.tile_pool(name="dummy", bufs=1) as dp:
                        dummy = dp.tile([P, E], F32, name="dummy")
                        nc.gpsimd.memset(dummy[:], 0.0)
                        for m in range(NM):
                            nc.sync.dma_start(
                                y_ext.ap()[m * P:(m + 1) * P, :],
                                dummy[:])
                if do_p4:
                    ot_ctx.__exit__(None, None, None)
                sc_ctx.__exit__(None, None, None)

    nc.compile()
    nc.m = get_hw_module(nc.m)
    return nc


def _prep_in_maps(x, W_qkv, b_qkv, W_out, b_out):
    x = np.asarray(x, dtype=np.float32)
    W_qkv = np.asarray(W_qkv, dtype=np.float32)
    b_qkv = np.asarray(b_qkv, dtype=np.float32)
    W_out = np.asarray(W_out, dtype=np.float32)
    b_out = np.asarray(b_out, dtype=np.float32)

    xT = np.ascontiguousarray(x.T)                       # [E, S]
    # W_qk tiled: [16 g][16 k][128 p][256 c] with g = m-pair group
    w_qk = W_qkv[:, :2 * E].reshape(NE, P, 16, 256).transpose(2, 0, 1, 3)
    w_qk = np.ascontiguousarray(w_qk)
    w_v = np.ascontiguousarray(W_qkv[:, 2 * E:])         # [E, E]
    b_qk_t = np.ascontiguousarray(
        np.broadcast_to(b_qkv[:2 * E].reshape(2 * NE, P).T, (P, 2 * NE)))
    b_v = b_qkv[2 * E:]
    b_eff_row = b_v @ W_out + b_out                      # [E]
    b_eff = np.ascontiguousarray(
        np.broadcast_to(b_eff_row[None, :], (P, E))).astype(np.float32)
    col_iota = (np.arange(SL)[None, :] - np.arange(P)[:, None]
                ).astype(np.float32)                     # f - p

    in_maps = []
    for r in range(N_CORES):
        thr_r = np.broadcast_to(
            (P * np.arange(NS, dtype=np.float32) - SL * r)[None, :],
            (P, NS))
        in_maps.append({
            "xT": np.ascontiguousarray(xT[:, r * SL:(r + 1) * SL]),
            "w_qk": w_qk,
            "w_v": w_v,
            "w_out": W_out,
            "b_qk": b_qk_t,
            "b_eff": b_eff,
            "col_iota": col_iota,
            "ones": np.ones((P, 256), dtype=np.float32),
            "thr": np.ascontiguousarray(thr_r),
        })
    return in_maps


def get_nc():
    if "nc" not in _CACHE:
        _CACHE["nc"] = _build()
    return _CACHE["nc"]


def kernel(x, W_qkv, b_qkv, W_out, b_out):
    nc = get_nc()
    in_maps = _prep_in_maps(x, W_qkv, b_qkv, W_out, b_out)
    res = run_bass_kernel_spmd(nc, in_maps, core_ids=list(range(N_CORES)))
    y = np.concatenate([res.results[r]["y"] for r in range(N_CORES)], axis=0)
    return y.astype(np.float32)

